# revision 17
# baseline (speedup 1.0000x reference)
"""Distributed statevector Hadamard-gate kernel for 8 TRN2 NeuronCores.

Problem: y = U @ x where U = kron_{i=0..23}(M if i in (0,5,10,15,20) else I2),
x is a 2^24-amplitude complex64 statevector (qudit 0 = most significant axis),
M is the 2x2 Hadamard (real-valued).

Strategy
--------
M is real, so real/imag parts transform independently -> treat x as a float
stream (interleaved re,im; bit-strides of qubit axes double).  The rel-err
budget (2e-2) dwarfs fp16 rounding (~1e-3), so all HBM traffic is fp16:
half the bytes of fp32 -> half the memory-roofline time.

Shard across 8 cores by qubits (1,2,3) (non-gate axes) -> every gate is local
to a core; no collectives.  While sharding, the host also permutes qubit axes
so each core's 2^22-elem fp16 slab has bit layout

  s = [q0 q4 q5 q9 q10 q14 q15 | q6 q7 q8 q11 q12 | q13 q16..q19 q20 q21..q23 reim]
       '------ partition -----' '---- chunk -----' '------- chunk free -------'

Partition index = (q0 q4 q5 q9 q10 q14 q15): gates on q0,q5,q10,q15 become ONE
128x128 fp16 matmul with W = kron(M,I,M,I,M,I,M) (q20's scale folded in), and
every DMA is plain 2D with 2KB contiguous lines.  The q20 gate is a fp16
add/sub butterfly on the vector engine (2x 16-bit rate) before the matmul;
PSUM is evacuated fp32->fp16 by the scalar engine.  Single HBM pass,
pipelined over 32 x 256KB chunks per core; DMA is the bottleneck engine.
"""

import math
import sys
import types

import numpy as np

import concourse.bass as bass
import concourse.mybir as mybir
from concourse.tile import TileContext
from concourse.bass_utils import run_bass_kernel_spmd


def _ensure_axon_hooks():
    """bass_utils' trace path does `from antenv.axon_hooks import ...`
    unconditionally; some images ship an `antenv` without that submodule,
    which would crash tracing.  Synthesize it (and register the ctypes NTFF
    hook when available) so tracing degrades gracefully instead.
    """
    try:
        import antenv.axon_hooks  # noqa: F401

        return
    except ImportError:
        pass
    try:
        import antenv
    except ImportError:
        return
    mod = types.ModuleType("antenv.axon_hooks")
    mod._hook = None

    def set_axon_ntff_profile_hook(hook):
        mod._hook = hook

    def get_axon_ntff_profile_hook():
        return mod._hook

    mod.set_axon_ntff_profile_hook = set_axon_ntff_profile_hook
    mod.get_axon_ntff_profile_hook = get_axon_ntff_profile_hook
    sys.modules["antenv.axon_hooks"] = mod
    antenv.axon_hooks = mod
    try:
        from trn_agent_boot.trn_boot import _ntff_profile_via_ctypes

        hook = _ntff_profile_via_ctypes("/opt/axon/libaxon_pjrt.so")
        if hook is not None:
            mod._hook = hook
    except Exception:
        pass


_ensure_axon_hooks()


def _legalize_waits(bir: dict) -> dict:
    """This image's walrus accepts only ONE sync-wait per TPB/DMA
    instruction; Tile emits up to ~4.  Hoist all but the last wait of each
    instruction into standalone EventSemaphore ops on the same engine,
    placed immediately before it — semantically identical (the engine
    blocks on them in program order).
    """
    for f in bir.get("functions", []):
        for b in f.get("blocks", []):
            out = []
            for i in b["instructions"]:
                si = i.get("sync_info") or {}
                waits = si.get("on_wait") or []
                if len(waits) > 1:
                    for k, wt in enumerate(waits[:-1]):
                        out.append({
                            "debug": i.get("debug", 0),
                            "engine": i["engine"],
                            "ins": [], "outs": [],
                            "name": f"hoistwait_{i['name']}_{k}",
                            "opcode": "EventSemaphore",
                            "sync_info": {"on_update": [], "on_wait": [wt]},
                        })
                    si["on_wait"] = [waits[-1]]
                out.append(i)
            b["instructions"] = out
    return bir


def _install_legalizer():
    import json as _json

    orig = bass.Bass.to_json_bytes
    if getattr(bass.Bass, "_wait_legalizer_installed", False):
        return

    def to_json_bytes(self, *a, **kw):
        raw = orig(self, *a, **kw)
        try:
            return _json.dumps(_legalize_waits(_json.loads(raw))).encode()
        except Exception:
            return raw

    bass.Bass.to_json_bytes = to_json_bytes
    bass.Bass._wait_legalizer_installed = True


_install_legalizer()

N_CORES = 8

_NC_CACHE: dict = {}

# set by kernel(): the BassKernelResults of the last run (exec_time_ns when
# run with BASS_TRACE=1) — used by the local test harness only
LAST_RESULT = None


def _build_nc(S: int, bfly):
    """Build the SPMD Bass program for one core.

    S: log2 of per-core slab element count (22 for complex64 input).
    bfly: ("had",) for add/sub butterfly (scale folded into W), or
          ("gen", a, b, c, d) for a generic real 2x2 q20 gate.
    """
    RUN = 1 << (S - 13)  # matmul N / PSUM bank: 512 (cplx) / 256
    CHUNK_FREE = 4 * RUN  # per-partition free elems per chunk
    NCHUNKS = 16
    HALF = CHUNK_FREE // 2
    L_SUB = RUN // 32  # q21..q23[,reim] size below the q20 bit
    f16 = mybir.dt.float16
    fp32 = mybir.dt.float32

    nc = bass.Bass()
    x = nc.declare_dram_parameter("x", [1 << S], f16, isOutput=False)
    w = nc.declare_dram_parameter("w", [128, 128], f16, isOutput=False)
    y = nc.declare_dram_parameter("y", [1 << S], f16, isOutput=True)

    GRP = 2  # chunks per in-DMA: groups are contiguous in the free dim, so
    # a group in-DMA is plain 2D with 8KB lines; the DGE stripes one DMA's
    # lines across all 16 queues -> full aggregate BW from very few triggers

    # slab = [p(7) | t(4) | f], p = partition, t = chunk, f = chunk free.
    xv = x[:].rearrange("(p g f) -> g p f", p=128, g=NCHUNKS // GRP,
                        f=GRP * CHUNK_FREE)
    yv = y[:].rearrange("(p t f) -> t p f", p=128, t=NCHUNKS, f=CHUNK_FREE)

    with TileContext(nc) as tc:
        with (
            tc.tile_pool(name="wpool", bufs=1) as wpool,
            tc.tile_pool(name="inp", bufs=5) as inp,
            tc.tile_pool(name="bfp", bufs=4) as bfp,
            tc.tile_pool(name="outp", bufs=4) as outp,
            tc.tile_pool(name="psp", bufs=8, space="PSUM") as psp,
        ):
            wts = wpool.tile([128, 128], f16, tag="wstage")
            nc.sync.dma_start(out=wts[:], in_=w[:])
            # stage via DVE so matmuls' weight dep is on the DVE semaphore
            wt = wpool.tile([128, 128], f16, tag="wmain")
            nc.vector.tensor_copy(wt[:], wts[:])

            # ~600ns of sequencer time per dma_start trigger caps the DMA
            # issue rate, so: few triggers, out triggers split sync/scalar.
            # Only the first 4 group in-DMAs are issued up front; later ones
            # are interleaved so ins don't hog the HBM port while outs starve.
            NGRP = NCHUNKS // GRP
            its = []

            def emit_in(g):
                it = inp.tile([128, GRP * CHUNK_FREE], f16)
                nc.sync.dma_start(out=it[:], in_=xv[g])
                its.append(it)

            for g in range(5):
                emit_in(g)

            for t in range(NCHUNKS):
                it = its[t // GRP]
                cf = slice((t % GRP) * CHUNK_FREE, (t % GRP + 1) * CHUNK_FREE)
                # q20 butterfly in fp16 (2x DVE rate):
                # free = (m, q20, low) = (m, 2, L_SUB)
                bf = bfp.tile([128, CHUNK_FREE], f16)
                mm = CHUNK_FREE // (2 * L_SUB)
                iv = it[:, cf].rearrange("p (m w l) -> p m w l", m=mm, w=2, l=L_SUB)
                bv = bf[:].rearrange("p (m w l) -> p m w l", m=mm, w=2, l=L_SUB)
                _bfly_pair(
                    nc.vector, mybir, bfly,
                    bv[:, :, 0, :], bv[:, :, 1, :], iv[:, :, 0, :], iv[:, :, 1, :],
                )

                # gates on q0,q5,q10,q15 = one 128x128 matmul on the partition
                # dim, tiled over PSUM banks; evacuation split scalar/vector
                # evacuation balance: scalar does 3 copies on even chunks,
                # 2 on odd (vector also runs the butterfly)
                nsc = 3 if t % 2 == 0 else 2
                ot = outp.tile([128, CHUNK_FREE], f16)
                for j in range(CHUNK_FREE // RUN):
                    sl = slice(j * RUN, (j + 1) * RUN)
                    ps = psp.tile([128, RUN], fp32)
                    nc.tensor.matmul(ps[:], wt[:], bf[:, sl], start=True, stop=True)
                    if j < nsc:
                        nc.scalar.copy(ot[:, sl], ps[:])
                    else:
                        nc.vector.tensor_copy(ot[:, sl], ps[:])

                # release the next in-group before the out trigger so the
                # in never queues behind a compute-dependent out wait
                g = (t + 9) // GRP
                if t % GRP == 1 and g < NGRP:
                    emit_in(g)

                if t % 2 == 0:
                    nc.sync.dma_start(out=yv[t], in_=ot[:])
                else:
                    nc.scalar.dma_start(out=yv[t], in_=ot[:])
    return nc


def _bfly_pair(eng, mb, bfly, out0, out1, i0, i1):
    """Apply a 2x2 gate to the (i0, i1) pair of equally-shaped views."""
    if bfly[0] == "had":
        eng.tensor_add(out0, i0, i1)
        eng.tensor_sub(out1, i0, i1)
    else:
        _, ga, gb, gc, gd = bfly
        # out0 = ga*x0 + gb*x1 ; out1 = gc*x0 + gd*x1
        eng.tensor_scalar_mul(out0, i0, float(ga))
        eng.scalar_tensor_tensor(
            out0, i1, float(gb), out0, mb.AluOpType.mult, mb.AluOpType.add
        )
        eng.tensor_scalar_mul(out1, i0, float(gc))
        eng.scalar_tensor_tensor(
            out1, i1, float(gd), out1, mb.AluOpType.mult, mb.AluOpType.add
        )


def _get_nc(S: int, bfly):
    key = (S, bfly)
    if key not in _NC_CACHE:
        _NC_CACHE[key] = _build_nc(S, bfly)
    return _NC_CACHE[key]


def _build_W(Mr: np.ndarray, fold_scale: float) -> np.ndarray:
    """128x128 real matrix applying M on partition bits q0, q5, q10, q15.

    Partition index p = (q0 q4 q5 q9 q10 q14 q15), MSB first.
    """
    I2 = np.eye(2, dtype=np.float64)
    W = np.array([[1.0]])
    for F in (Mr, I2, Mr, I2, Mr, I2, Mr):  # q0, q4, q5, q9, q10, q14, q15
        W = np.kron(W, F)
    return W * fold_scale


def kernel(x: np.ndarray, M: np.ndarray) -> np.ndarray:
    x = np.asarray(x)
    M = np.asarray(M)
    n, batch = x.shape
    assert n == 1 << 24 and batch == 1, (n, batch)

    is_complex = np.iscomplexobj(x)
    if is_complex:
        xc = np.ascontiguousarray(x, dtype=np.complex64)
        xf = xc.reshape(-1).view(np.float32)
    else:
        xf = np.ascontiguousarray(x, dtype=np.float32).reshape(-1)
    F = xf.size
    S = int(round(math.log2(F))) - 3  # per-core slab elems = F/8
    FD = F >> 16  # contiguous tail (q16..q23[,reim]): 512 (cplx) / 256

    # gate matrix: must be (essentially) real
    Mc = np.asarray(M, dtype=np.complex128)
    assert np.abs(Mc.imag).max() <= 1e-5 * max(np.abs(Mc.real).max(), 1e-30), (
        "complex-valued M is not supported"
    )
    Mr = Mc.real.copy()

    s0 = Mr[0, 0]
    had_form = (
        abs(s0) > 0
        and abs(Mr[0, 1] - s0) <= 1e-6 * abs(s0)
        and abs(Mr[1, 0] - s0) <= 1e-6 * abs(s0)
        and abs(Mr[1, 1] + s0) <= 1e-6 * abs(s0)
    )
    if had_form:
        bfly = ("had",)
        W = _build_W(Mr, fold_scale=s0)  # q20's unnormalized butterfly scale
    else:
        bfly = ("gen", Mr[0, 0], Mr[0, 1], Mr[1, 0], Mr[1, 1])
        W = _build_W(Mr, fold_scale=1.0)
    wT = np.ascontiguousarray(W.T).astype(np.float16)  # lhsT[k, i] = W[i, k]

    nc = _get_nc(S, bfly)

    # fp16 + shard by qubits (1,2,3) + permute (q9 q10),(q14 q15) up into the
    # partition bits.  Full-array dims, MSB->LSB:
    #   (q0, q1q2q3, q4q5, q6q7q8, q9q10, q11q12q13, q14q15, tail)
    xh = xf.astype(np.float16)
    xt = xh.reshape(2, 8, 4, 8, 4, 8, 4, FD).transpose(1, 0, 2, 4, 6, 3, 5, 7)
    xs = np.ascontiguousarray(xt)  # (core, q0, q4q5, q9q10, q14q15, q6q7q8, q11q12q13, tail)
    in_maps = [
        {"x": xs[cid].reshape(-1), "w": wT} for cid in range(N_CORES)
    ]
    res = run_bass_kernel_spmd(nc, in_maps, list(range(N_CORES)))
    global LAST_RESULT
    LAST_RESULT = res
    outs = res.results

    yt = np.empty((8, 2, 4, 4, 4, 8, 8, FD), dtype=np.float16)
    for cid in range(N_CORES):
        yt[cid] = outs[cid]["y"].reshape(2, 4, 4, 4, 8, 8, FD)
    # inverse permute + upcast
    yf = yt.transpose(1, 0, 2, 5, 3, 6, 4, 7).astype(np.float32).reshape(F)

    if is_complex:
        return yf.view(np.complex64).reshape(n, batch)
    return yf.reshape(n, batch)


# revision 18
# speedup vs baseline: 1.0684x; 1.0684x over previous
"""Distributed statevector Hadamard-gate kernel for 8 TRN2 NeuronCores.

Problem: y = U @ x where U = kron_{i=0..23}(M if i in (0,5,10,15,20) else I2),
x is a 2^24-amplitude complex64 statevector (qudit 0 = most significant axis),
M is the 2x2 Hadamard (real-valued).

Strategy
--------
M is real, so real/imag parts transform independently -> treat x as a float
stream (interleaved re,im; bit-strides of qubit axes double).  The rel-err
budget (2e-2) dwarfs fp16 rounding (~1e-3), so all HBM traffic is fp16:
half the bytes of fp32 -> half the memory-roofline time.

Shard across 8 cores by qubits (1,2,3) (non-gate axes) -> every gate is local
to a core; no collectives.  While sharding, the host also permutes qubit axes
so each core's 2^22-elem fp16 slab has bit layout

  s = [q0 q4 q5 q9 q10 q14 q15 | q6 q7 q8 q11 q12 | q13 q16..q19 q20 q21..q23 reim]
       '------ partition -----' '---- chunk -----' '------- chunk free -------'

Partition index = (q0 q4 q5 q9 q10 q14 q15): gates on q0,q5,q10,q15 become ONE
128x128 fp16 matmul with W = kron(M,I,M,I,M,I,M) (q20's scale folded in), and
every DMA is plain 2D with 2KB contiguous lines.  The q20 gate is a fp16
add/sub butterfly on the vector engine (2x 16-bit rate) before the matmul;
PSUM is evacuated fp32->fp16 by the scalar engine.  Single HBM pass,
pipelined over 32 x 256KB chunks per core; DMA is the bottleneck engine.
"""

import math
import sys
import types

import numpy as np

import concourse.bass as bass
import concourse.mybir as mybir
from concourse.tile import TileContext
from concourse.bass_utils import run_bass_kernel_spmd


def _ensure_axon_hooks():
    """bass_utils' trace path does `from antenv.axon_hooks import ...`
    unconditionally; some images ship an `antenv` without that submodule,
    which would crash tracing.  Synthesize it (and register the ctypes NTFF
    hook when available) so tracing degrades gracefully instead.
    """
    try:
        import antenv.axon_hooks  # noqa: F401

        return
    except ImportError:
        pass
    try:
        import antenv
    except ImportError:
        return
    mod = types.ModuleType("antenv.axon_hooks")
    mod._hook = None

    def set_axon_ntff_profile_hook(hook):
        mod._hook = hook

    def get_axon_ntff_profile_hook():
        return mod._hook

    mod.set_axon_ntff_profile_hook = set_axon_ntff_profile_hook
    mod.get_axon_ntff_profile_hook = get_axon_ntff_profile_hook
    sys.modules["antenv.axon_hooks"] = mod
    antenv.axon_hooks = mod
    try:
        from trn_agent_boot.trn_boot import _ntff_profile_via_ctypes

        hook = _ntff_profile_via_ctypes("/opt/axon/libaxon_pjrt.so")
        if hook is not None:
            mod._hook = hook
    except Exception:
        pass


_ensure_axon_hooks()


def _legalize_waits(bir: dict) -> dict:
    """This image's walrus accepts only ONE sync-wait per TPB/DMA
    instruction; Tile emits up to ~4.  Hoist all but the last wait of each
    instruction into standalone EventSemaphore ops on the same engine,
    placed immediately before it — semantically identical (the engine
    blocks on them in program order).
    """
    for f in bir.get("functions", []):
        for b in f.get("blocks", []):
            out = []
            for i in b["instructions"]:
                si = i.get("sync_info") or {}
                waits = si.get("on_wait") or []
                if len(waits) > 1:
                    for k, wt in enumerate(waits[:-1]):
                        out.append({
                            "debug": i.get("debug", 0),
                            "engine": i["engine"],
                            "ins": [], "outs": [],
                            "name": f"hoistwait_{i['name']}_{k}",
                            "opcode": "EventSemaphore",
                            "sync_info": {"on_update": [], "on_wait": [wt]},
                        })
                    si["on_wait"] = [waits[-1]]
                out.append(i)
            b["instructions"] = out
    return bir


def _install_legalizer():
    import json as _json

    orig = bass.Bass.to_json_bytes
    if getattr(bass.Bass, "_wait_legalizer_installed", False):
        return

    def to_json_bytes(self, *a, **kw):
        raw = orig(self, *a, **kw)
        try:
            return _json.dumps(_legalize_waits(_json.loads(raw))).encode()
        except Exception:
            return raw

    bass.Bass.to_json_bytes = to_json_bytes
    bass.Bass._wait_legalizer_installed = True


_install_legalizer()

N_CORES = 8

_NC_CACHE: dict = {}

# set by kernel(): the BassKernelResults of the last run (exec_time_ns when
# run with BASS_TRACE=1) — used by the local test harness only
LAST_RESULT = None


def _build_nc(S: int, bfly):
    """Build the SPMD Bass program for one core.

    S: log2 of per-core slab element count (22 for complex64 input).
    bfly: ("had",) for add/sub butterfly (scale folded into W), or
          ("gen", a, b, c, d) for a generic real 2x2 q20 gate.
    """
    RUN = 1 << (S - 13)  # matmul N / PSUM bank: 512 (cplx) / 256
    CHUNK_FREE = 4 * RUN  # per-partition free elems per chunk
    NCHUNKS = 16
    HALF = CHUNK_FREE // 2
    L_SUB = RUN // 32  # q21..q23[,reim] size below the q20 bit
    f16 = mybir.dt.float16
    fp32 = mybir.dt.float32

    nc = bass.Bass()
    x = nc.declare_dram_parameter("x", [1 << S], f16, isOutput=False)
    w = nc.declare_dram_parameter("w", [128, 128], f16, isOutput=False)
    y = nc.declare_dram_parameter("y", [1 << S], f16, isOutput=True)

    GRP = 2  # chunks per in-DMA: groups are contiguous in the free dim, so
    # a group in-DMA is plain 2D with 8KB lines; the DGE stripes one DMA's
    # lines across all 16 queues -> full aggregate BW from very few triggers

    # slab = [p(7) | t(4) | f], p = partition, t = chunk, f = chunk free.
    xv = x[:].rearrange("(p g f) -> g p f", p=128, g=NCHUNKS // GRP,
                        f=GRP * CHUNK_FREE)
    yv = y[:].rearrange("(p t f) -> t p f", p=128, t=NCHUNKS, f=CHUNK_FREE)

    with TileContext(nc) as tc:
        with (
            tc.tile_pool(name="wpool", bufs=1) as wpool,
            tc.tile_pool(name="inp", bufs=4) as inp,
            tc.tile_pool(name="bfp", bufs=4) as bfp,
            tc.tile_pool(name="outp", bufs=4) as outp,
            tc.tile_pool(name="psp", bufs=8, space="PSUM") as psp,
        ):
            wts = wpool.tile([128, 128], f16, tag="wstage")
            nc.sync.dma_start(out=wts[:], in_=w[:])
            # stage via DVE so matmuls' weight dep is on the DVE semaphore
            wt = wpool.tile([128, 128], f16, tag="wmain")
            nc.vector.tensor_copy(wt[:], wts[:])

            # ~600ns of sequencer time per dma_start trigger caps the DMA
            # issue rate, so: few triggers, out triggers split sync/scalar.
            # Only the first 4 group in-DMAs are issued up front; later ones
            # are interleaved so ins don't hog the HBM port while outs starve.
            NGRP = NCHUNKS // GRP
            its = []

            def emit_in(g):
                it = inp.tile([128, GRP * CHUNK_FREE], f16)
                nc.sync.dma_start(out=it[:], in_=xv[g])
                its.append(it)

            for g in range(4):
                emit_in(g)

            for t in range(NCHUNKS):
                it = its[t // GRP]
                cf = slice((t % GRP) * CHUNK_FREE, (t % GRP + 1) * CHUNK_FREE)
                # q20 butterfly in fp16 (2x DVE rate):
                # free = (m, q20, low) = (m, 2, L_SUB)
                bf = bfp.tile([128, CHUNK_FREE], f16)
                mm = CHUNK_FREE // (2 * L_SUB)
                iv = it[:, cf].rearrange("p (m w l) -> p m w l", m=mm, w=2, l=L_SUB)
                bv = bf[:].rearrange("p (m w l) -> p m w l", m=mm, w=2, l=L_SUB)
                _bfly_pair(
                    nc.vector, mybir, bfly,
                    bv[:, :, 0, :], bv[:, :, 1, :], iv[:, :, 0, :], iv[:, :, 1, :],
                )

                # gates on q0,q5,q10,q15 = one 128x128 matmul on the partition
                # dim, tiled over PSUM banks; evacuation split scalar/vector
                # evacuation balance: scalar does 3 copies on even chunks,
                # 2 on odd (vector also runs the butterfly)
                nsc = 3
                ot = outp.tile([128, CHUNK_FREE], f16)
                for j in range(CHUNK_FREE // RUN):
                    sl = slice(j * RUN, (j + 1) * RUN)
                    ps = psp.tile([128, RUN], fp32)
                    nc.tensor.matmul(ps[:], wt[:], bf[:, sl], start=True, stop=True)
                    if j < nsc:
                        nc.scalar.copy(ot[:, sl], ps[:])
                    else:
                        nc.vector.tensor_copy(ot[:, sl], ps[:])

                # release the next in-group before the out trigger so the
                # in never queues behind a compute-dependent out wait
                g = (t + 7) // GRP
                if t % GRP == 1 and g < NGRP:
                    emit_in(g)

                if t % 2 == 0:
                    nc.sync.dma_start(out=yv[t], in_=ot[:])
                else:
                    nc.scalar.dma_start(out=yv[t], in_=ot[:])
    return nc


def _bfly_pair(eng, mb, bfly, out0, out1, i0, i1):
    """Apply a 2x2 gate to the (i0, i1) pair of equally-shaped views."""
    if bfly[0] == "had":
        eng.tensor_add(out0, i0, i1)
        eng.tensor_sub(out1, i0, i1)
    else:
        _, ga, gb, gc, gd = bfly
        # out0 = ga*x0 + gb*x1 ; out1 = gc*x0 + gd*x1
        eng.tensor_scalar_mul(out0, i0, float(ga))
        eng.scalar_tensor_tensor(
            out0, i1, float(gb), out0, mb.AluOpType.mult, mb.AluOpType.add
        )
        eng.tensor_scalar_mul(out1, i0, float(gc))
        eng.scalar_tensor_tensor(
            out1, i1, float(gd), out1, mb.AluOpType.mult, mb.AluOpType.add
        )


def _get_nc(S: int, bfly):
    key = (S, bfly)
    if key not in _NC_CACHE:
        _NC_CACHE[key] = _build_nc(S, bfly)
    return _NC_CACHE[key]


def _build_W(Mr: np.ndarray, fold_scale: float) -> np.ndarray:
    """128x128 real matrix applying M on partition bits q0, q5, q10, q15.

    Partition index p = (q0 q4 q5 q9 q10 q14 q15), MSB first.
    """
    I2 = np.eye(2, dtype=np.float64)
    W = np.array([[1.0]])
    for F in (Mr, I2, Mr, I2, Mr, I2, Mr):  # q0, q4, q5, q9, q10, q14, q15
        W = np.kron(W, F)
    return W * fold_scale


def kernel(x: np.ndarray, M: np.ndarray) -> np.ndarray:
    x = np.asarray(x)
    M = np.asarray(M)
    n, batch = x.shape
    assert n == 1 << 24 and batch == 1, (n, batch)

    is_complex = np.iscomplexobj(x)
    if is_complex:
        xc = np.ascontiguousarray(x, dtype=np.complex64)
        xf = xc.reshape(-1).view(np.float32)
    else:
        xf = np.ascontiguousarray(x, dtype=np.float32).reshape(-1)
    F = xf.size
    S = int(round(math.log2(F))) - 3  # per-core slab elems = F/8
    FD = F >> 16  # contiguous tail (q16..q23[,reim]): 512 (cplx) / 256

    # gate matrix: must be (essentially) real
    Mc = np.asarray(M, dtype=np.complex128)
    assert np.abs(Mc.imag).max() <= 1e-5 * max(np.abs(Mc.real).max(), 1e-30), (
        "complex-valued M is not supported"
    )
    Mr = Mc.real.copy()

    s0 = Mr[0, 0]
    had_form = (
        abs(s0) > 0
        and abs(Mr[0, 1] - s0) <= 1e-6 * abs(s0)
        and abs(Mr[1, 0] - s0) <= 1e-6 * abs(s0)
        and abs(Mr[1, 1] + s0) <= 1e-6 * abs(s0)
    )
    if had_form:
        bfly = ("had",)
        W = _build_W(Mr, fold_scale=s0)  # q20's unnormalized butterfly scale
    else:
        bfly = ("gen", Mr[0, 0], Mr[0, 1], Mr[1, 0], Mr[1, 1])
        W = _build_W(Mr, fold_scale=1.0)
    wT = np.ascontiguousarray(W.T).astype(np.float16)  # lhsT[k, i] = W[i, k]

    nc = _get_nc(S, bfly)

    # fp16 + shard by qubits (1,2,3) + permute (q9 q10),(q14 q15) up into the
    # partition bits.  Full-array dims, MSB->LSB:
    #   (q0, q1q2q3, q4q5, q6q7q8, q9q10, q11q12q13, q14q15, tail)
    xh = xf.astype(np.float16)
    xt = xh.reshape(2, 8, 4, 8, 4, 8, 4, FD).transpose(1, 0, 2, 4, 6, 3, 5, 7)
    xs = np.ascontiguousarray(xt)  # (core, q0, q4q5, q9q10, q14q15, q6q7q8, q11q12q13, tail)
    in_maps = [
        {"x": xs[cid].reshape(-1), "w": wT} for cid in range(N_CORES)
    ]
    res = run_bass_kernel_spmd(nc, in_maps, list(range(N_CORES)))
    global LAST_RESULT
    LAST_RESULT = res
    outs = res.results

    yt = np.empty((8, 2, 4, 4, 4, 8, 8, FD), dtype=np.float16)
    for cid in range(N_CORES):
        yt[cid] = outs[cid]["y"].reshape(2, 4, 4, 4, 8, 8, FD)
    # inverse permute + upcast
    yf = yt.transpose(1, 0, 2, 5, 3, 6, 4, 7).astype(np.float32).reshape(F)

    if is_complex:
        return yf.view(np.complex64).reshape(n, batch)
    return yf.reshape(n, batch)


# revision 19
# speedup vs baseline: 1.0978x; 1.0275x over previous
"""Distributed statevector Hadamard-gate kernel for 8 TRN2 NeuronCores.

Problem: y = U @ x where U = kron_{i=0..23}(M if i in (0,5,10,15,20) else I2),
x is a 2^24-amplitude complex64 statevector (qudit 0 = most significant axis),
M is the 2x2 Hadamard (real-valued).

Strategy
--------
M is real, so real/imag parts transform independently -> treat x as a float
stream (interleaved re,im; bit-strides of qubit axes double).  The rel-err
budget (2e-2) dwarfs fp16 rounding (~1e-3), so all HBM traffic is fp16:
half the bytes of fp32 -> half the memory-roofline time.

Shard across 8 cores by qubits (1,2,3) (non-gate axes) -> every gate is local
to a core; no collectives.  While sharding, the host also permutes qubit axes
so each core's 2^22-elem fp16 slab has bit layout

  s = [q0 q4 q5 q9 q10 q14 q15 | q6 q7 q8 q11 q12 | q13 q16..q19 q20 q21..q23 reim]
       '------ partition -----' '---- chunk -----' '------- chunk free -------'

Partition index = (q0 q4 q5 q9 q10 q14 q15): gates on q0,q5,q10,q15 become ONE
128x128 fp16 matmul with W = kron(M,I,M,I,M,I,M) (q20's scale folded in), and
every DMA is plain 2D with 2KB contiguous lines.  The q20 gate is a fp16
add/sub butterfly on the vector engine (2x 16-bit rate) before the matmul;
PSUM is evacuated fp32->fp16 by the scalar engine.  Single HBM pass,
pipelined over 32 x 256KB chunks per core; DMA is the bottleneck engine.
"""

import math
import sys
import types

import numpy as np

import concourse.bass as bass
import concourse.mybir as mybir
from concourse.tile import TileContext
from concourse.bass_utils import run_bass_kernel_spmd


def _ensure_axon_hooks():
    """bass_utils' trace path does `from antenv.axon_hooks import ...`
    unconditionally; some images ship an `antenv` without that submodule,
    which would crash tracing.  Synthesize it (and register the ctypes NTFF
    hook when available) so tracing degrades gracefully instead.
    """
    try:
        import antenv.axon_hooks  # noqa: F401

        return
    except ImportError:
        pass
    try:
        import antenv
    except ImportError:
        return
    mod = types.ModuleType("antenv.axon_hooks")
    mod._hook = None

    def set_axon_ntff_profile_hook(hook):
        mod._hook = hook

    def get_axon_ntff_profile_hook():
        return mod._hook

    mod.set_axon_ntff_profile_hook = set_axon_ntff_profile_hook
    mod.get_axon_ntff_profile_hook = get_axon_ntff_profile_hook
    sys.modules["antenv.axon_hooks"] = mod
    antenv.axon_hooks = mod
    try:
        from trn_agent_boot.trn_boot import _ntff_profile_via_ctypes

        hook = _ntff_profile_via_ctypes("/opt/axon/libaxon_pjrt.so")
        if hook is not None:
            mod._hook = hook
    except Exception:
        pass


_ensure_axon_hooks()


def _legalize_waits(bir: dict) -> dict:
    """This image's walrus accepts only ONE sync-wait per TPB/DMA
    instruction; Tile emits up to ~4.  Hoist all but the last wait of each
    instruction into standalone EventSemaphore ops on the same engine,
    placed immediately before it — semantically identical (the engine
    blocks on them in program order).
    """
    for f in bir.get("functions", []):
        for b in f.get("blocks", []):
            out = []
            for i in b["instructions"]:
                si = i.get("sync_info") or {}
                waits = si.get("on_wait") or []
                if len(waits) > 1:
                    for k, wt in enumerate(waits[:-1]):
                        out.append({
                            "debug": i.get("debug", 0),
                            "engine": i["engine"],
                            "ins": [], "outs": [],
                            "name": f"hoistwait_{i['name']}_{k}",
                            "opcode": "EventSemaphore",
                            "sync_info": {"on_update": [], "on_wait": [wt]},
                        })
                    si["on_wait"] = [waits[-1]]
                out.append(i)
            b["instructions"] = out
    return bir


def _install_legalizer():
    import json as _json

    orig = bass.Bass.to_json_bytes
    if getattr(bass.Bass, "_wait_legalizer_installed", False):
        return

    def to_json_bytes(self, *a, **kw):
        raw = orig(self, *a, **kw)
        try:
            return _json.dumps(_legalize_waits(_json.loads(raw))).encode()
        except Exception:
            return raw

    bass.Bass.to_json_bytes = to_json_bytes
    bass.Bass._wait_legalizer_installed = True


_install_legalizer()

N_CORES = 8

_NC_CACHE: dict = {}

# set by kernel(): the BassKernelResults of the last run (exec_time_ns when
# run with BASS_TRACE=1) — used by the local test harness only
LAST_RESULT = None


def _build_nc(S: int, bfly):
    """Build the SPMD Bass program for one core.

    S: log2 of per-core slab element count (22 for complex64 input).
    bfly: ("had",) for add/sub butterfly (scale folded into W), or
          ("gen", a, b, c, d) for a generic real 2x2 q20 gate.
    """
    RUN = 1 << (S - 13)  # matmul N / PSUM bank: 512 (cplx) / 256
    CHUNK_FREE = 4 * RUN  # per-partition free elems per chunk
    NCHUNKS = 16
    HALF = CHUNK_FREE // 2
    L_SUB = RUN // 32  # q21..q23[,reim] size below the q20 bit
    f16 = mybir.dt.float16
    fp32 = mybir.dt.float32

    nc = bass.Bass()
    x = nc.declare_dram_parameter("x", [1 << S], f16, isOutput=False)
    w = nc.declare_dram_parameter("w", [128, 128], f16, isOutput=False)
    y = nc.declare_dram_parameter("y", [1 << S], f16, isOutput=True)

    GRP = 2  # chunks per in-DMA: groups are contiguous in the free dim, so
    # a group in-DMA is plain 2D with 8KB lines; the DGE stripes one DMA's
    # lines across all 16 queues -> full aggregate BW from very few triggers

    # slab = [p(7) | t(4) | f], p = partition, t = chunk, f = chunk free.
    xv = x[:].rearrange("(p g f) -> g p f", p=128, g=NCHUNKS // GRP,
                        f=GRP * CHUNK_FREE)
    yv = y[:].rearrange("(p t f) -> t p f", p=128, t=NCHUNKS, f=CHUNK_FREE)

    with TileContext(nc) as tc:
        with (
            tc.tile_pool(name="wpool", bufs=1) as wpool,
            tc.tile_pool(name="inp", bufs=4) as inp,
            tc.tile_pool(name="bfp", bufs=4) as bfp,
            tc.tile_pool(name="outp", bufs=4) as outp,
            tc.tile_pool(name="psp", bufs=8, space="PSUM") as psp,
        ):
            wts = wpool.tile([128, 128], f16, tag="wstage")
            nc.sync.dma_start(out=wts[:], in_=w[:])
            # stage via DVE so matmuls' weight dep is on the DVE semaphore
            wt = wpool.tile([128, 128], f16, tag="wmain")
            nc.vector.tensor_copy(wt[:], wts[:])

            # ~600ns of sequencer time per dma_start trigger caps the DMA
            # issue rate, so: few triggers, out triggers split sync/scalar.
            # Only the first 4 group in-DMAs are issued up front; later ones
            # are interleaved so ins don't hog the HBM port while outs starve.
            NGRP = NCHUNKS // GRP
            its = []

            def emit_in(g):
                it = inp.tile([128, GRP * CHUNK_FREE], f16)
                nc.sync.dma_start(out=it[:], in_=xv[g])
                its.append(it)

            for g in range(4):
                emit_in(g)

            for t in range(NCHUNKS):
                it = its[t // GRP]
                cf = slice((t % GRP) * CHUNK_FREE, (t % GRP + 1) * CHUNK_FREE)
                # q20 butterfly in fp16 (2x DVE rate):
                # free = (m, q20, low) = (m, 2, L_SUB)
                bf = bfp.tile([128, CHUNK_FREE], f16)
                mm = CHUNK_FREE // (2 * L_SUB)
                iv = it[:, cf].rearrange("p (m w l) -> p m w l", m=mm, w=2, l=L_SUB)
                bv = bf[:].rearrange("p (m w l) -> p m w l", m=mm, w=2, l=L_SUB)
                _bfly_pair(
                    nc.vector, mybir, bfly,
                    bv[:, :, 0, :], bv[:, :, 1, :], iv[:, :, 0, :], iv[:, :, 1, :],
                )

                # gates on q0,q5,q10,q15 = one 128x128 matmul on the partition
                # dim, tiled over PSUM banks; evacuation split scalar/vector
                # evacuation balance: scalar does 3 copies on even chunks,
                # 2 on odd (vector also runs the butterfly)
                nsc = 3
                ot = outp.tile([128, CHUNK_FREE], f16)
                for j in range(CHUNK_FREE // RUN):
                    sl = slice(j * RUN, (j + 1) * RUN)
                    ps = psp.tile([128, RUN], fp32)
                    nc.tensor.matmul(ps[:], wt[:], bf[:, sl], start=True, stop=True)
                    if j < nsc:
                        nc.scalar.copy(ot[:, sl], ps[:])
                    else:
                        nc.vector.tensor_copy(ot[:, sl], ps[:])

                if t % 2 == 0:
                    nc.sync.dma_start(out=yv[t], in_=ot[:])
                else:
                    nc.scalar.dma_start(out=yv[t], in_=ot[:])

                # release the next in-group once this group's last reader
                # has been emitted (pool bufs=4 enforces the data hazard)
                g = (t + 7) // GRP
                if t % GRP == 1 and g < NGRP:
                    emit_in(g)
    return nc


def _bfly_pair(eng, mb, bfly, out0, out1, i0, i1):
    """Apply a 2x2 gate to the (i0, i1) pair of equally-shaped views."""
    if bfly[0] == "had":
        eng.tensor_add(out0, i0, i1)
        eng.tensor_sub(out1, i0, i1)
    else:
        _, ga, gb, gc, gd = bfly
        # out0 = ga*x0 + gb*x1 ; out1 = gc*x0 + gd*x1
        eng.tensor_scalar_mul(out0, i0, float(ga))
        eng.scalar_tensor_tensor(
            out0, i1, float(gb), out0, mb.AluOpType.mult, mb.AluOpType.add
        )
        eng.tensor_scalar_mul(out1, i0, float(gc))
        eng.scalar_tensor_tensor(
            out1, i1, float(gd), out1, mb.AluOpType.mult, mb.AluOpType.add
        )


def _get_nc(S: int, bfly):
    key = (S, bfly)
    if key not in _NC_CACHE:
        _NC_CACHE[key] = _build_nc(S, bfly)
    return _NC_CACHE[key]


def _build_W(Mr: np.ndarray, fold_scale: float) -> np.ndarray:
    """128x128 real matrix applying M on partition bits q0, q5, q10, q15.

    Partition index p = (q0 q4 q5 q9 q10 q14 q15), MSB first.
    """
    I2 = np.eye(2, dtype=np.float64)
    W = np.array([[1.0]])
    for F in (Mr, I2, Mr, I2, Mr, I2, Mr):  # q0, q4, q5, q9, q10, q14, q15
        W = np.kron(W, F)
    return W * fold_scale


def kernel(x: np.ndarray, M: np.ndarray) -> np.ndarray:
    x = np.asarray(x)
    M = np.asarray(M)
    n, batch = x.shape
    assert n == 1 << 24 and batch == 1, (n, batch)

    is_complex = np.iscomplexobj(x)
    if is_complex:
        xc = np.ascontiguousarray(x, dtype=np.complex64)
        xf = xc.reshape(-1).view(np.float32)
    else:
        xf = np.ascontiguousarray(x, dtype=np.float32).reshape(-1)
    F = xf.size
    S = int(round(math.log2(F))) - 3  # per-core slab elems = F/8
    FD = F >> 16  # contiguous tail (q16..q23[,reim]): 512 (cplx) / 256

    # gate matrix: must be (essentially) real
    Mc = np.asarray(M, dtype=np.complex128)
    assert np.abs(Mc.imag).max() <= 1e-5 * max(np.abs(Mc.real).max(), 1e-30), (
        "complex-valued M is not supported"
    )
    Mr = Mc.real.copy()

    s0 = Mr[0, 0]
    had_form = (
        abs(s0) > 0
        and abs(Mr[0, 1] - s0) <= 1e-6 * abs(s0)
        and abs(Mr[1, 0] - s0) <= 1e-6 * abs(s0)
        and abs(Mr[1, 1] + s0) <= 1e-6 * abs(s0)
    )
    if had_form:
        bfly = ("had",)
        W = _build_W(Mr, fold_scale=s0)  # q20's unnormalized butterfly scale
    else:
        bfly = ("gen", Mr[0, 0], Mr[0, 1], Mr[1, 0], Mr[1, 1])
        W = _build_W(Mr, fold_scale=1.0)
    wT = np.ascontiguousarray(W.T).astype(np.float16)  # lhsT[k, i] = W[i, k]

    nc = _get_nc(S, bfly)

    # fp16 + shard by qubits (1,2,3) + permute (q9 q10),(q14 q15) up into the
    # partition bits.  Full-array dims, MSB->LSB:
    #   (q0, q1q2q3, q4q5, q6q7q8, q9q10, q11q12q13, q14q15, tail)
    xh = xf.astype(np.float16)
    xt = xh.reshape(2, 8, 4, 8, 4, 8, 4, FD).transpose(1, 0, 2, 4, 6, 3, 5, 7)
    xs = np.ascontiguousarray(xt)  # (core, q0, q4q5, q9q10, q14q15, q6q7q8, q11q12q13, tail)
    in_maps = [
        {"x": xs[cid].reshape(-1), "w": wT} for cid in range(N_CORES)
    ]
    res = run_bass_kernel_spmd(nc, in_maps, list(range(N_CORES)))
    global LAST_RESULT
    LAST_RESULT = res
    outs = res.results

    yt = np.empty((8, 2, 4, 4, 4, 8, 8, FD), dtype=np.float16)
    for cid in range(N_CORES):
        yt[cid] = outs[cid]["y"].reshape(2, 4, 4, 4, 8, 8, FD)
    # inverse permute + upcast
    yf = yt.transpose(1, 0, 2, 5, 3, 6, 4, 7).astype(np.float32).reshape(F)

    if is_complex:
        return yf.view(np.complex64).reshape(n, batch)
    return yf.reshape(n, batch)


# revision 22
# speedup vs baseline: 1.1327x; 1.0318x over previous
"""Distributed statevector Hadamard-gate kernel for 8 TRN2 NeuronCores.

Problem: y = U @ x where U = kron_{i=0..23}(M if i in (0,5,10,15,20) else I2),
x is a 2^24-amplitude complex64 statevector (qudit 0 = most significant axis),
M is the 2x2 Hadamard (real-valued).

Strategy
--------
M is real, so real/imag parts transform independently -> treat x as a float
stream (interleaved re,im; bit-strides of qubit axes double).  The rel-err
budget (2e-2) dwarfs fp16 rounding (~1e-3), so all HBM traffic is fp16:
half the bytes of fp32 -> half the memory-roofline time.

Shard across 8 cores by qubits (1,2,3) (non-gate axes) -> every gate is local
to a core; no collectives.  While sharding, the host also permutes qubit axes
so each core's 2^22-elem fp16 slab has bit layout

  s = [q0 q4 q5 q9 q10 q14 q15 | q6 q7 q8 q11 q12 | q13 q16..q19 q20 q21..q23 reim]
       '------ partition -----' '---- chunk -----' '------- chunk free -------'

Partition index = (q0 q4 q5 q9 q10 q14 q15): gates on q0,q5,q10,q15 become ONE
128x128 fp16 matmul with W = kron(M,I,M,I,M,I,M) (q20's scale folded in), and
every DMA is plain 2D with 2KB contiguous lines.  The q20 gate is a fp16
add/sub butterfly on the vector engine (2x 16-bit rate) before the matmul;
PSUM is evacuated fp32->fp16 by the scalar engine.  Single HBM pass,
pipelined over 32 x 256KB chunks per core; DMA is the bottleneck engine.
"""

import math
import sys
import types

import numpy as np

import concourse.bass as bass
import concourse.mybir as mybir
from concourse.tile import TileContext
from concourse.bass_utils import run_bass_kernel_spmd


def _ensure_axon_hooks():
    """bass_utils' trace path does `from antenv.axon_hooks import ...`
    unconditionally; some images ship an `antenv` without that submodule,
    which would crash tracing.  Synthesize it (and register the ctypes NTFF
    hook when available) so tracing degrades gracefully instead.
    """
    try:
        import antenv.axon_hooks  # noqa: F401

        return
    except ImportError:
        pass
    try:
        import antenv
    except ImportError:
        return
    mod = types.ModuleType("antenv.axon_hooks")
    mod._hook = None

    def set_axon_ntff_profile_hook(hook):
        mod._hook = hook

    def get_axon_ntff_profile_hook():
        return mod._hook

    mod.set_axon_ntff_profile_hook = set_axon_ntff_profile_hook
    mod.get_axon_ntff_profile_hook = get_axon_ntff_profile_hook
    sys.modules["antenv.axon_hooks"] = mod
    antenv.axon_hooks = mod
    try:
        from trn_agent_boot.trn_boot import _ntff_profile_via_ctypes

        hook = _ntff_profile_via_ctypes("/opt/axon/libaxon_pjrt.so")
        if hook is not None:
            mod._hook = hook
    except Exception:
        pass


_ensure_axon_hooks()


def _legalize_waits(bir: dict) -> dict:
    """This image's walrus accepts only ONE sync-wait per TPB/DMA
    instruction; Tile emits up to ~4.  Hoist all but the last wait of each
    instruction into standalone EventSemaphore ops on the same engine,
    placed immediately before it — semantically identical (the engine
    blocks on them in program order).
    """
    for f in bir.get("functions", []):
        for b in f.get("blocks", []):
            out = []
            for i in b["instructions"]:
                si = i.get("sync_info") or {}
                waits = si.get("on_wait") or []
                if len(waits) > 1:
                    for k, wt in enumerate(waits[:-1]):
                        out.append({
                            "debug": i.get("debug", 0),
                            "engine": i["engine"],
                            "ins": [], "outs": [],
                            "name": f"hoistwait_{i['name']}_{k}",
                            "opcode": "EventSemaphore",
                            "sync_info": {"on_update": [], "on_wait": [wt]},
                        })
                    si["on_wait"] = [waits[-1]]
                out.append(i)
            b["instructions"] = out
    return bir


def _install_legalizer():
    import json as _json

    orig = bass.Bass.to_json_bytes
    if getattr(bass.Bass, "_wait_legalizer_installed", False):
        return

    def to_json_bytes(self, *a, **kw):
        raw = orig(self, *a, **kw)
        try:
            return _json.dumps(_legalize_waits(_json.loads(raw))).encode()
        except Exception:
            return raw

    bass.Bass.to_json_bytes = to_json_bytes
    bass.Bass._wait_legalizer_installed = True


_install_legalizer()

N_CORES = 8

_NC_CACHE: dict = {}

# set by kernel(): the BassKernelResults of the last run (exec_time_ns when
# run with BASS_TRACE=1) — used by the local test harness only
LAST_RESULT = None


def _build_nc(S: int, bfly):
    """Build the SPMD Bass program for one core.

    S: log2 of per-core slab element count (22 for complex64 input).
    bfly: ("had",) for add/sub butterfly (scale folded into W), or
          ("gen", a, b, c, d) for a generic real 2x2 q20 gate.
    """
    RUN = 1 << (S - 13)  # matmul N / PSUM bank: 512 (cplx) / 256
    CHUNK_FREE = 4 * RUN  # per-partition free elems per chunk
    NCHUNKS = 16
    HALF = CHUNK_FREE // 2
    L_SUB = RUN // 32  # q21..q23[,reim] size below the q20 bit
    f16 = mybir.dt.float16
    fp32 = mybir.dt.float32

    nc = bass.Bass()
    x = nc.declare_dram_parameter("x", [1 << S], f16, isOutput=False)
    w = nc.declare_dram_parameter("w", [128, 128], f16, isOutput=False)
    y = nc.declare_dram_parameter("y", [1 << S], f16, isOutput=True)

    GRP = 2  # chunks per in-DMA: groups are contiguous in the free dim, so
    # a group in-DMA is plain 2D with 8KB lines; the DGE stripes one DMA's
    # lines across all 16 queues -> full aggregate BW from very few triggers

    # slab = [p(7) | t(4) | f], p = partition, t = chunk, f = chunk free.
    xv = x[:].rearrange("(p g f) -> g p f", p=128, g=NCHUNKS // GRP,
                        f=GRP * CHUNK_FREE)
    yv = y[:].rearrange("(p t f) -> t p f", p=128, t=NCHUNKS, f=CHUNK_FREE)

    with TileContext(nc) as tc:
        with (
            tc.tile_pool(name="wpool", bufs=1) as wpool,
            tc.tile_pool(name="inp", bufs=4) as inp,
            tc.tile_pool(name="bfp", bufs=4) as bfp,
            tc.tile_pool(name="outp", bufs=4) as outp,
            tc.tile_pool(name="psp", bufs=8, space="PSUM") as psp,
        ):
            wts = wpool.tile([128, 128], f16, tag="wstage")
            nc.sync.dma_start(out=wts[:], in_=w[:])
            # stage via DVE so matmuls' weight dep is on the DVE semaphore
            wt = wpool.tile([128, 128], f16, tag="wmain")
            nc.vector.tensor_copy(wt[:], wts[:])

            # ~600ns of sequencer time per dma_start trigger caps the DMA
            # issue rate, so: few triggers, out triggers split sync/scalar.
            # Only the first 4 group in-DMAs are issued up front; later ones
            # are interleaved so ins don't hog the HBM port while outs starve.
            NGRP = NCHUNKS // GRP
            its = []

            def emit_in(g, eng=None):
                it = inp.tile([128, GRP * CHUNK_FREE], f16)
                (eng or nc.sync).dma_start(out=it[:], in_=xv[g])
                its.append(it)

            # upfront ins split across both sequencers: port saturates
            # ~2x sooner (scalar is idle until the first matmul lands)
            for g in range(4):
                emit_in(g, nc.scalar if g % 2 else nc.sync)

            for t in range(NCHUNKS):
                it = its[t // GRP]
                cf = slice((t % GRP) * CHUNK_FREE, (t % GRP + 1) * CHUNK_FREE)
                # q20 butterfly in fp16 (2x DVE rate):
                # free = (m, q20, low) = (m, 2, L_SUB)
                bf = bfp.tile([128, CHUNK_FREE], f16)
                mc = CHUNK_FREE // (2 * L_SUB)
                iv = it[:, cf].rearrange("p (m w l) -> p m w l", m=mc, w=2, l=L_SUB)
                bv = bf[:].rearrange("p (m w l) -> p m w l", m=mc, w=2, l=L_SUB)
                _bfly_pair(
                    nc.vector, mybir, bfly,
                    bv[:, :, 0, :], bv[:, :, 1, :], iv[:, :, 0, :], iv[:, :, 1, :],
                )

                # gates on q0,q5,q10,q15 = one 128x128 matmul on the partition
                # dim, tiled over PSUM banks; evacuation split scalar/vector
                ot = outp.tile([128, CHUNK_FREE], f16)
                for j in range(CHUNK_FREE // RUN):
                    sl = slice(j * RUN, (j + 1) * RUN)
                    ps = psp.tile([128, RUN], fp32)
                    nc.tensor.matmul(ps[:], wt[:], bf[:, sl], start=True, stop=True)
                    if j < 3:
                        nc.scalar.copy(ot[:, sl], ps[:])
                    else:
                        nc.vector.tensor_copy(ot[:, sl], ps[:])

                if t % 2 == 0:
                    nc.sync.dma_start(out=yv[t], in_=ot[:])
                else:
                    nc.scalar.dma_start(out=yv[t], in_=ot[:])

                # release the next in-group once this group's last reader
                # has been emitted (pool bufs=4 enforces the data hazard)
                g = (t + 7) // GRP
                if t % GRP == 1 and g < NGRP:
                    emit_in(g)
    return nc


def _bfly_pair(eng, mb, bfly, out0, out1, i0, i1):
    """Apply a 2x2 gate to the (i0, i1) pair of equally-shaped views."""
    if bfly[0] == "had":
        eng.tensor_add(out0, i0, i1)
        eng.tensor_sub(out1, i0, i1)
    else:
        _, ga, gb, gc, gd = bfly
        # out0 = ga*x0 + gb*x1 ; out1 = gc*x0 + gd*x1
        eng.tensor_scalar_mul(out0, i0, float(ga))
        eng.scalar_tensor_tensor(
            out0, i1, float(gb), out0, mb.AluOpType.mult, mb.AluOpType.add
        )
        eng.tensor_scalar_mul(out1, i0, float(gc))
        eng.scalar_tensor_tensor(
            out1, i1, float(gd), out1, mb.AluOpType.mult, mb.AluOpType.add
        )


def _get_nc(S: int, bfly):
    key = (S, bfly)
    if key not in _NC_CACHE:
        _NC_CACHE[key] = _build_nc(S, bfly)
    return _NC_CACHE[key]


def _build_W(Mr: np.ndarray, fold_scale: float) -> np.ndarray:
    """128x128 real matrix applying M on partition bits q0, q5, q10, q15.

    Partition index p = (q0 q4 q5 q9 q10 q14 q15), MSB first.
    """
    I2 = np.eye(2, dtype=np.float64)
    W = np.array([[1.0]])
    for F in (Mr, I2, Mr, I2, Mr, I2, Mr):  # q0, q4, q5, q9, q10, q14, q15
        W = np.kron(W, F)
    return W * fold_scale


def kernel(x: np.ndarray, M: np.ndarray) -> np.ndarray:
    x = np.asarray(x)
    M = np.asarray(M)
    n, batch = x.shape
    assert n == 1 << 24 and batch == 1, (n, batch)

    is_complex = np.iscomplexobj(x)
    if is_complex:
        xc = np.ascontiguousarray(x, dtype=np.complex64)
        xf = xc.reshape(-1).view(np.float32)
    else:
        xf = np.ascontiguousarray(x, dtype=np.float32).reshape(-1)
    F = xf.size
    S = int(round(math.log2(F))) - 3  # per-core slab elems = F/8
    FD = F >> 16  # contiguous tail (q16..q23[,reim]): 512 (cplx) / 256

    # gate matrix: must be (essentially) real
    Mc = np.asarray(M, dtype=np.complex128)
    assert np.abs(Mc.imag).max() <= 1e-5 * max(np.abs(Mc.real).max(), 1e-30), (
        "complex-valued M is not supported"
    )
    Mr = Mc.real.copy()

    s0 = Mr[0, 0]
    had_form = (
        abs(s0) > 0
        and abs(Mr[0, 1] - s0) <= 1e-6 * abs(s0)
        and abs(Mr[1, 0] - s0) <= 1e-6 * abs(s0)
        and abs(Mr[1, 1] + s0) <= 1e-6 * abs(s0)
    )
    if had_form:
        bfly = ("had",)
        W = _build_W(Mr, fold_scale=s0)  # q20's unnormalized butterfly scale
    else:
        bfly = ("gen", Mr[0, 0], Mr[0, 1], Mr[1, 0], Mr[1, 1])
        W = _build_W(Mr, fold_scale=1.0)
    wT = np.ascontiguousarray(W.T).astype(np.float16)  # lhsT[k, i] = W[i, k]

    nc = _get_nc(S, bfly)

    # fp16 + shard by qubits (1,2,3) + permute (q9 q10),(q14 q15) up into the
    # partition bits.  Full-array dims, MSB->LSB:
    #   (q0, q1q2q3, q4q5, q6q7q8, q9q10, q11q12q13, q14q15, tail)
    xh = xf.astype(np.float16)
    xt = xh.reshape(2, 8, 4, 8, 4, 8, 4, FD).transpose(1, 0, 2, 4, 6, 3, 5, 7)
    xs = np.ascontiguousarray(xt)  # (core, q0, q4q5, q9q10, q14q15, q6q7q8, q11q12q13, tail)
    in_maps = [
        {"x": xs[cid].reshape(-1), "w": wT} for cid in range(N_CORES)
    ]
    res = run_bass_kernel_spmd(nc, in_maps, list(range(N_CORES)))
    global LAST_RESULT
    LAST_RESULT = res
    outs = res.results

    yt = np.empty((8, 2, 4, 4, 4, 8, 8, FD), dtype=np.float16)
    for cid in range(N_CORES):
        yt[cid] = outs[cid]["y"].reshape(2, 4, 4, 4, 8, 8, FD)
    # inverse permute + upcast
    yf = yt.transpose(1, 0, 2, 5, 3, 6, 4, 7).astype(np.float32).reshape(F)

    if is_complex:
        return yf.view(np.complex64).reshape(n, batch)
    return yf.reshape(n, batch)
